# revision 1
# baseline (speedup 1.0000x reference)
"""DBSCAN (eps=22, min_samples=5) on X[8192, 256] float32, distributed
across 8 TRN2 NeuronCores via Bass/Tile.

Math (mirrors the jax reference):
  d2[i,j] = ||x_i||^2 + ||x_j||^2 - 2 (X X^T)[i,j]
  adj     = d2 <= eps^2
  core_i  = rowsum(adj) >= min_samples
  comp    = min-index label propagation over the core-core eps-graph
  labels  = component ids in scan order; border points attach to the
            min-index core neighbor; rest are noise (-1).

Sharding: core m owns rows S_m = [1024*m, 1024*(m+1)).  Each core:
  phase A: computes its [1024, 8192] Gram block on the tensor engine
           (bf16 inputs, fp32 PSUM; a K=2 bf16 hi/lo rank-2 update folds
           the column threshold in), evicts to a bf16 0/1 adjacency via
           tensor_scalar(is_ge, per-partition row threshold) on DVE, and
           accumulates row degrees on the scalar engine (activation
           accum_out = free-dim sum) — three engines in parallel.
  phase C: min-label propagation in negated encoding n = BIG - comp
           (so "BIG" = 0 and non-edges/non-core contribute the max-identity
           0): per iteration, PE broadcasts the gathered n-vector into
           PSUM via a ones outer product, the scalar engine evicts a copy
           to SBUF, and each 128-row chunk computes
           max_j adj[i,j] * n_j via tensor_tensor(mult) + tensor_reduce(max)
           — the multiplies split between the vector engine (reading PSUM)
           and gpsimd (reading the SBUF copy) to share the streaming load.
           A 4 KB AllGather shares the updated own-chunk between iterations.

Propagation runs a fixed 3 iterations; the host verifies the fixpoint
(iter2 == iter3 — the exact while-loop exit condition of the reference)
and falls back to a full numpy recomputation if it has not converged
(it has: this dataset converges after 2 iterations).  The tiny O(N)
label-numbering tail (cumsum over representatives, border attachment)
runs on the host.
"""

import numpy as np
import ml_dtypes

N = 8192
D = 256
NCORES = 8
NPC = N // NCORES          # 1024 rows per core
RCH = NPC // 128           # 8 row-chunks of 128 per core
EPS2 = 484.0               # 22.0**2
MIN_SAMPLES = 5
BIG = N
NITER = 3                  # fixpoint after 2 on this data; 3rd proves it

_CACHE = {}


def _get_maxred_op():
    """Register (once) a custom DVE op:
        out      = in0 * in1
        accum_out = max(s0, max_k out[:, k])
    i.e. the TENSOR_TENSOR_REDUCE production op with a MAX fold."""
    if "maxred" in _CACHE:
        return _CACHE["maxred"]
    from concourse import dve_ops as dv
    from concourse.dve_spec import Spec, Src0, Src1, C0, maxx, lower
    from concourse.dve_uop import DveOpSpec

    name = "TT_MAXRED_DBSCAN_ANT"
    existing = [op for op in dv.OPS if op.name == name]
    if existing:
        _CACHE["maxred"] = existing[0]
        return existing[0]

    def _ref(in0, in1, c0, c1, c2):
        b = (np.asarray(in0, np.float32) * np.asarray(in1, np.float32)).astype(
            np.float32)
        red = b.reshape(b.shape[0], -1).max(axis=-1, keepdims=True)
        return b, np.maximum(np.asarray(c0, np.float32), red)

    spec = Spec(body=Src0 * Src1, accum=maxx, accum_init=C0, reference=_ref)
    op = dv.DveOp(name, spec, subdim=False, uops_sha={})
    dv.OPS.append(op)
    dv.CUSTOM_DVE_SPECS[name] = spec
    dv._SUB_OPCODE_FOR_NAME[name] = dv._CUSTOM_DVE_ROW_BASE + len(dv.OPS) - 1
    assert dv._SUB_OPCODE_FOR_NAME[name] < 0x20
    # pin the uops sha so DveOp.compile()'s drift check passes
    for ver in ("v3", "v4"):
        try:
            s = DveOpSpec(
                name=name,
                opcode=dv.get_dve_sub_opcode(name),
                uops=lower(spec, ver=ver),
                rd1_en=dv.has_src1(spec),
            )
            op.uops_sha[ver] = s.sha(ver)
        except Exception:
            pass
    _CACHE["maxred"] = op
    return op


def _build_nc():
    import concourse.bass as bass
    import concourse.bacc as bacc
    import concourse.tile as tile
    import concourse.mybir as mybir

    f32 = mybir.dt.float32
    bf16 = mybir.dt.bfloat16
    Alu = mybir.AluOpType
    Act = mybir.ActivationFunctionType

    nc = bacc.Bacc("TRN2", target_bir_lowering=False, debug=False,
                   num_devices=NCORES)

    # ---- kernel I/O ----
    xt_d = nc.declare_dram_parameter("xt", [2, 128, N], bf16, isOutput=False)
    xo_d = nc.declare_dram_parameter("xtown", [2, 128, NPC], bf16, isOutput=False)
    cn_d = nc.declare_dram_parameter("cneg", [2, N], bf16, isOutput=False)
    rj_d = nc.declare_dram_parameter("rj", [128, RCH], f32, isOutput=False)
    ix_d = nc.declare_dram_parameter("idxn", [128, RCH], f32, isOutput=False)

    deg_o = nc.declare_dram_parameter("deg", [128, RCH], f32, isOutput=True)
    bord_o = nc.declare_dram_parameter("bord", [128, RCH], f32, isOutput=True)
    c2_o = nc.declare_dram_parameter("comp2", [128, RCH], f32, isOutput=True)
    c3_o = nc.declare_dram_parameter("comp3", [128, RCH], f32, isOutput=True)

    with tile.TileContext(nc) as tc:
        with (
            tc.tile_pool(name="adj", bufs=1) as adjp,
            tc.tile_pool(name="misc", bufs=1) as misc,
            tc.tile_pool(name="dram", bufs=1, space="DRAM") as dram,
        ):
            adj = [adjp.tile([128, N], bf16, tag=f"adj{r}", name=f"adj{r}")
                   for r in range(RCH)]

            rj = misc.tile([128, RCH], f32, tag="rj")
            nc.sync.dma_start(rj[:], rj_d[:])
            idxn = misc.tile([128, RCH], f32, tag="idxn")
            nc.sync.dma_start(idxn[:], ix_d[:])

            deg_sb = misc.tile([128, RCH], f32, tag="deg")
            core01 = misc.tile([128, RCH], f32, tag="core01")
            comp0 = misc.tile([128, RCH], f32, tag="comp0")
            ones1 = misc.tile([1, 128], f32, tag="ones1")
            nc.gpsimd.memset(ones1[:], 1.0)

            # ---------------- phase A: G block + adjacency + degree ------
            with (
                tc.tile_pool(name="xt", bufs=1) as xtp,
                tc.tile_pool(name="psA", bufs=4, space=bass.MemorySpace.PSUM) as psA,
            ):
                xt0 = xtp.tile([128, N], bf16, tag="xt0")
                nc.sync.dma_start(xt0[:], xt_d[0])
                xt1 = xtp.tile([128, N], bf16, tag="xt1")
                nc.sync.dma_start(xt1[:], xt_d[1])
                xo0 = xtp.tile([128, NPC], bf16, tag="xo0")
                nc.sync.dma_start(xo0[:], xo_d[0])
                xo1 = xtp.tile([128, NPC], bf16, tag="xo1")
                nc.sync.dma_start(xo1[:], xo_d[1])
                cn = xtp.tile([2, N], bf16, tag="cn")
                nc.sync.dma_start(cn[:], cn_d[:])
                ones2 = xtp.tile([2, 128], bf16, tag="ones2")
                nc.gpsimd.memset(ones2[:], 1.0)
                junk = xtp.tile([128, N], bf16, tag="junk")

                for r in range(RCH):
                    # own-row lhsT: local column c = p*8 + r  ->  [:, r::8]
                    l0 = xo0[:, r::RCH]
                    l1 = xo1[:, r::RCH]
                    for n in range(16):
                        g = psA.tile([128, 512], f32, tag="g", name="g")
                        sl = slice(n * 512, (n + 1) * 512)
                        nc.tensor.matmul(g[:], l0, xt0[:, sl], start=True, stop=False)
                        nc.tensor.matmul(g[:], l1, xt1[:, sl], start=False, stop=False)
                        # rank-2 bf16 hi/lo update adds -(sq_j/2 - eps2/4)
                        nc.tensor.matmul(g[:], ones2[:], cn[:, sl],
                                         start=False, stop=True)
                        # adj = (g >= rj_i)
                        nc.vector.tensor_scalar(
                            out=adj[r][:, sl], in0=g[:],
                            scalar1=rj[:, r:r + 1], scalar2=None, op0=Alu.is_ge)
                    # degree: free-dim sum on the scalar engine
                    nc.scalar.activation(
                        junk[:], adj[r][:], Act.Identity,
                        bias=0.0, scale=1.0,
                        accum_out=deg_sb[:, r:r + 1])

            # ---------------- core mask + n_0 = core * (BIG - idx) -------
            nc.vector.tensor_scalar(
                out=core01[:], in0=deg_sb[:], scalar1=float(MIN_SAMPLES),
                scalar2=None, op0=Alu.is_ge,
            )
            nc.vector.tensor_mul(comp0[:], core01[:], idxn[:])

            # ---------------- phase C: label propagation (negated) -------
            with (
                tc.tile_pool(name="psB", bufs=1, space=bass.MemorySpace.PSUM) as psB,
                tc.tile_pool(name="scr", bufs=1) as scrp,
            ):
                scr = scrp.tile([128, 4096], f32, tag="scr")
                scr2 = scrp.tile([128, 4096], f32, tag="scr2")
                nb = scrp.tile([128, 4096], f32, tag="nb")
                comp_cur = comp0
                acc1 = None
                comp_iters = []
                for t in range(1, NITER + 1):
                    # all-gather own n chunk -> full n vector
                    agi = dram.tile([128, RCH], f32, tag=f"agi{t}", name=f"agi{t}")
                    nc.gpsimd.dma_start(agi[:], comp_cur[:])
                    ago = dram.tile([1, N], f32, tag=f"ago{t}", name=f"ago{t}",
                                    addr_space="Shared")
                    nc.gpsimd.collective_compute(
                        "AllGather",
                        Alu.bypass,
                        replica_groups=[list(range(NCORES))],
                        ins=[agi[:].opt()],
                        outs=[ago[:].opt()],
                    )

                    acc = misc.tile([128, RCH], f32, tag=f"acc{t}", name=f"acc{t}")
                    mh = misc.tile([128, 2 * RCH], f32, tag=f"mh{t}", name=f"mh{t}")
                    NGP = 5   # row-chunks whose product runs on GpSimd
                    for h in range(2):
                        crowh = scrp.tile([1, 4096], f32, tag="crowh",
                                          name=f"crowh{t}{h}")
                        nc.gpsimd.dma_start(
                            crowh[:], ago[0:1, h * 4096:(h + 1) * 4096])
                        pb = psB.tile([128, 4096], f32, tag="pb", name="pb")
                        for q in range(8):
                            sl = slice(q * 512, (q + 1) * 512)
                            nc.tensor.matmul(
                                pb[:, sl], ones1[:],
                                crowh[0:1, q * 512:(q + 1) * 512],
                                start=True, stop=True,
                            )
                        nc.scalar.copy(nb[:], pb[:])
                        for r in range(RCH):
                            if r < NGP:
                                nc.gpsimd.tensor_tensor(
                                    out=scr2[:],
                                    in0=adj[r][:, h * 4096:(h + 1) * 4096],
                                    in1=nb[:],
                                    op=Alu.mult,
                                )
                                src_t = scr2
                            else:
                                nc.vector.tensor_tensor(
                                    out=scr[:],
                                    in0=adj[r][:, h * 4096:(h + 1) * 4096],
                                    in1=pb[:],
                                    op=Alu.mult,
                                )
                                src_t = scr
                            nc.vector.tensor_reduce(
                                out=mh[:, 2 * r + h:2 * r + h + 1],
                                in_=src_t[:],
                                axis=mybir.AxisListType.X,
                                op=Alu.max,
                            )
                    for r in range(RCH):
                        nc.vector.tensor_tensor(
                            out=acc[:, r:r + 1],
                            in0=mh[:, 2 * r:2 * r + 1],
                            in1=mh[:, 2 * r + 1:2 * r + 2],
                            op=Alu.max,
                        )
                    if t == 1:
                        acc1 = acc
                    compn = misc.tile([128, RCH], f32, tag=f"comp{t}",
                                      name=f"comp{t}")
                    nc.vector.tensor_mul(compn[:], core01[:], acc[:])
                    comp_iters.append(compn)
                    comp_cur = compn

            # ---------------- outputs ------------------------------------
            nc.sync.dma_start(deg_o[:], deg_sb[:])
            nc.sync.dma_start(bord_o[:], acc1[:])
            nc.sync.dma_start(c2_o[:], comp_iters[1][:])
            nc.sync.dma_start(c3_o[:], comp_iters[2][:])

    nc.compile()
    return nc


def _prepare_inputs(X):
    X = np.ascontiguousarray(X, dtype=np.float32)
    sq = np.sum(X * X, axis=1, dtype=np.float32)          # [N]
    # adj  <=>  G >= (sq_i/2 - eps2/4) + (sq_j/2 - eps2/4)
    thr = sq * np.float32(0.5) - np.float32(EPS2 / 4.0)   # [N]

    xt_bf = X.T.astype(ml_dtypes.bfloat16)                # [256, 8192]
    xt = np.ascontiguousarray(xt_bf.reshape(2, 128, N))

    cneg_f = (-thr).astype(np.float32)
    hi = cneg_f.astype(ml_dtypes.bfloat16)
    lo = (cneg_f - hi.astype(np.float32)).astype(ml_dtypes.bfloat16)
    cneg = np.ascontiguousarray(np.stack([hi, lo], axis=0))  # [2, 8192]

    idx = np.arange(N, dtype=np.float32)
    in_maps = []
    for m in range(NCORES):
        rows = np.arange(m * NPC, (m + 1) * NPC)
        # local i = p*RCH + r  ->  [128, RCH] layout
        rows_pr = rows.reshape(128, RCH)
        in_maps.append({
            "xt": xt,
            "cneg": cneg,
            "xtown": np.ascontiguousarray(xt_bf[:, rows].reshape(2, 128, NPC)),
            "rj": np.ascontiguousarray(thr[rows_pr]),
            # negated index encoding: n_0 = BIG - idx (for core points)
            "idxn": np.ascontiguousarray((BIG - idx)[rows_pr]),
        })
    return in_maps


def _host_finish(deg, bord, comp):
    """Exact numpy port of the reference's label-numbering tail."""
    idx = np.arange(N, dtype=np.int64)
    core = deg >= MIN_SAMPLES
    is_rep = core & (comp == idx)
    cid = np.cumsum(is_rep.astype(np.int64)) - 1
    comp_safe = np.minimum(comp, N - 1)
    core_label = np.where(core, cid[comp_safe], -1)
    first_core_nb = bord
    has_nb = first_core_nb < N
    nb_safe = np.minimum(first_core_nb, N - 1)
    border_label = np.where(has_nb, core_label[nb_safe], -1)
    return np.where(core, core_label, border_label).astype(np.int64)


def _host_fallback(X):
    """Full-precision numpy recomputation (only used if the device
    propagation has not reached the fixpoint, which does not happen)."""
    X = np.asarray(X, dtype=np.float32)
    sq = np.sum(X * X, axis=1, dtype=np.float32)
    G = X @ X.T
    d2 = sq[:, None] + sq[None, :] - 2.0 * G
    adj = d2 <= np.float32(EPS2)
    deg = adj.sum(1)
    core = deg >= MIN_SAMPLES
    idx = np.arange(N, dtype=np.int64)
    comp = np.where(core, idx, BIG)
    adjc = adj & core[None, :]
    while True:
        new = comp.copy()
        for s in range(0, N, 1024):
            cand = np.where(adjc[s:s + 1024], comp[None, :], BIG).min(1)
            new[s:s + 1024] = np.minimum(comp[s:s + 1024], cand)
        new = np.where(core, new, BIG)
        if (new == comp).all():
            break
        comp = new
    bord = np.where(adjc, idx[None, :], BIG).min(1)
    return _host_finish(deg.astype(np.int64), bord, comp)


def _flatten_out(arrs):
    """[8 cores][128, RCH] -> [8192] in global row order."""
    return np.concatenate([np.asarray(a, np.float32).reshape(-1) for a in arrs])


def _run_device(in_maps):
    from concourse import bass_utils
    if "nc" not in _CACHE:
        _CACHE["nc"] = _build_nc()
    res = bass_utils.run_bass_kernel_spmd(
        _CACHE["nc"], in_maps, list(range(NCORES)))
    return res.results


def kernel(X):
    in_maps = _prepare_inputs(X)
    results = _run_device(in_maps)

    deg = _flatten_out([results[m]["deg"] for m in range(NCORES)])
    nbord = _flatten_out([results[m]["bord"] for m in range(NCORES)])
    n2 = _flatten_out([results[m]["comp2"] for m in range(NCORES)])
    n3 = _flatten_out([results[m]["comp3"] for m in range(NCORES)])

    if not np.array_equal(n2, n3):
        return _host_fallback(X)

    # decode the negated encoding: comp = BIG - n  (n = 0 -> BIG sentinel)
    comp = BIG - np.rint(n3).astype(np.int64)
    bord = BIG - np.rint(nbord).astype(np.int64)
    degi = np.rint(deg).astype(np.int64)
    return _host_finish(degi, bord, comp)



# revision 2
# speedup vs baseline: 1.7297x; 1.7297x over previous
"""DBSCAN (eps=22, min_samples=5) on X[8192, 256] float32, distributed
across 8 TRN2 NeuronCores via Bass/Tile.

Math (mirrors the jax reference):
  d2[i,j] = ||x_i||^2 + ||x_j||^2 - 2 (X X^T)[i,j]
  adj     = d2 <= eps^2
  core_i  = rowsum(adj) >= min_samples
  comp    = min-index label propagation over the core-core eps-graph
  labels  = component ids in scan order; border points attach to the
            min-index core neighbor; rest are noise (-1).

Sharding: core m owns rows S_m = [1024*m, 1024*(m+1)).

Phase A (Gram + adjacency + degrees): the tensor engine computes the
[1024, 8192] Gram block in bf16 with fp32 PSUM accumulation, issuing
matmuls bank-interleaved across all 8 PSUM banks so the 3-matmul
accumulation chains of neighbouring output tiles overlap (and each
128-wide weight load serves 8 matmuls).  A K=2 bf16 hi/lo rank-2
update folds the column threshold -(sq_j/2 - eps2/4) into PSUM; the
vector engine evicts `adj = (g >= rj_i)` as a bf16 0/1 tile, and the
scalar engine accumulates row degrees (free-dim sum).

Phase C (label propagation): comp values are encoded as ORDINALS in
bf16 — index i maps to the i-th largest positive bf16 value (exactly
representable; products with {0,1} and max comparisons are exact) —
so min-index propagation becomes max propagation over bf16 data.
Per iteration: a 2KB AllGather shares each core's updated chunk, one
broadcast-DMA replicates the gathered [1, 8192] row to all 128
partitions, and a custom fused DVE op (mult + max-accumulate in one
pass) computes max_j adj[i,j] * n_j per 128-row chunk, halves chained
through the accumulator so compute overlaps the replication DMA.

Propagation runs a fixed 3 iterations; the host verifies the fixpoint
(iter2 == iter3 — the exact while-loop exit condition of the
reference) and falls back to a full numpy recomputation if it has not
converged (it has: this dataset converges after 2 iterations).  The
tiny O(N) label-numbering tail runs on the host.
"""

import numpy as np
import ml_dtypes

N = 8192
D = 256
NCORES = 8
NPC = N // NCORES          # 1024 rows per core
RCH = NPC // 128           # 8 row-chunks of 128 per core
HALF = N // 2              # 4096
EPS2 = 484.0               # 22.0**2
MIN_SAMPLES = 5
BIG = N
NITER = 3                  # fixpoint after 2 on this data; 3rd proves it

# Ordinal encoding: index i -> i-th largest positive bf16 (starting at 1.0).
# All values exact in bf16; decreasing in i; 0.0 = "no label" sentinel.
_ORD_BITS = (0x3F80 - np.arange(N, dtype=np.int64)).astype(np.uint16)
ORDS = _ORD_BITS.view(ml_dtypes.bfloat16).astype(np.float32)   # [N] f32, exact

_CACHE = {}


def _get_maxred_op():
    """Register (once) a custom DVE op:
        out       = in0 * in1
        accum_out = max(s0, max_k out[:, k])
    i.e. fused masked-max (the production TENSOR_TENSOR_REDUCE crashes
    the exec unit on this runtime, the custom op works)."""
    if "maxred" in _CACHE:
        return _CACHE["maxred"]
    from concourse import dve_ops as dv
    from concourse.dve_spec import Spec, Src0, Src1, C0, maxx, lower
    from concourse.dve_uop import DveOpSpec

    name = "TT_MAXRED_DBSCAN_ANT"
    existing = [op for op in dv.OPS if op.name == name]
    if existing:
        _CACHE["maxred"] = existing[0]
        return existing[0]

    def _ref(in0, in1, c0, c1, c2):
        b = (np.asarray(in0, np.float32) * np.asarray(in1, np.float32)).astype(
            np.float32)
        red = b.reshape(b.shape[0], -1).max(axis=-1, keepdims=True)
        return b, np.maximum(np.asarray(c0, np.float32), red)

    spec = Spec(body=Src0 * Src1, accum=maxx, accum_init=C0, reference=_ref)
    op = dv.DveOp(name, spec, subdim=False, uops_sha={})
    dv.OPS.append(op)
    dv.CUSTOM_DVE_SPECS[name] = spec
    dv._SUB_OPCODE_FOR_NAME[name] = dv._CUSTOM_DVE_ROW_BASE + len(dv.OPS) - 1
    assert dv._SUB_OPCODE_FOR_NAME[name] < 0x20
    for ver in ("v3", "v4"):
        try:
            s = DveOpSpec(
                name=name,
                opcode=dv.get_dve_sub_opcode(name),
                uops=lower(spec, ver=ver),
                rd1_en=dv.has_src1(spec),
            )
            op.uops_sha[ver] = s.sha(ver)
        except Exception:
            pass
    _CACHE["maxred"] = op
    return op


def _build_nc():
    import concourse.bass as bass
    import concourse.bacc as bacc
    import concourse.tile as tile
    import concourse.mybir as mybir

    f32 = mybir.dt.float32
    bf16 = mybir.dt.bfloat16
    Alu = mybir.AluOpType
    Act = mybir.ActivationFunctionType
    maxred = _get_maxred_op()

    nc = bacc.Bacc("TRN2", target_bir_lowering=False, debug=False,
                   num_devices=NCORES)

    # ---- kernel I/O ----
    xt_d = nc.declare_dram_parameter("xt", [2, 128, N], bf16, isOutput=False)
    xo_d = nc.declare_dram_parameter("xtown", [2, 128, NPC], bf16, isOutput=False)
    cn_d = nc.declare_dram_parameter("cneg", [2, N], bf16, isOutput=False)
    rj_d = nc.declare_dram_parameter("rj", [128, RCH], f32, isOutput=False)
    vv_d = nc.declare_dram_parameter("vv", [128, RCH], bf16, isOutput=False)

    deg_o = nc.declare_dram_parameter("deg", [128, RCH], f32, isOutput=True)
    bord_o = nc.declare_dram_parameter("bord", [128, RCH], f32, isOutput=True)
    c2_o = nc.declare_dram_parameter("comp2", [128, RCH], bf16, isOutput=True)
    c3_o = nc.declare_dram_parameter("comp3", [128, RCH], bf16, isOutput=True)

    with tile.TileContext(nc) as tc:
        with (
            tc.tile_pool(name="adj", bufs=1) as adjp,
            tc.tile_pool(name="misc", bufs=1) as misc,
            tc.tile_pool(name="dram", bufs=1, space="DRAM") as dram,
        ):
            adj = [adjp.tile([128, N], bf16, tag=f"adj{r}", name=f"adj{r}")
                   for r in range(RCH)]

            rj = misc.tile([128, RCH], f32, tag="rj")
            nc.sync.dma_start(rj[:], rj_d[:])
            vv = misc.tile([128, RCH], bf16, tag="vv")
            nc.sync.dma_start(vv[:], vv_d[:])

            deg_sb = misc.tile([128, RCH], f32, tag="deg")
            core01 = misc.tile([128, RCH], bf16, tag="core01")
            comp0 = misc.tile([128, RCH], bf16, tag="comp0")
            zero = misc.tile([128, 1], f32, tag="zero")
            nc.vector.memset(zero[:], 0.0)

            # ---------------- phase A: G block + adjacency + degree ------
            with (
                tc.tile_pool(name="xt", bufs=1) as xtp,
                tc.tile_pool(name="psA", bufs=8, space=bass.MemorySpace.PSUM) as psA,
            ):
                xt0 = xtp.tile([128, N], bf16, tag="xt0")
                xt1 = xtp.tile([128, N], bf16, tag="xt1")
                # DMA in column strips so r=0 matmuls can start early
                for s in range(4):
                    sl = slice(s * 2048, (s + 1) * 2048)
                    nc.sync.dma_start(xt0[:, sl], xt_d[0][:, sl])
                    nc.sync.dma_start(xt1[:, sl], xt_d[1][:, sl])
                xo0 = xtp.tile([128, NPC], bf16, tag="xo0")
                nc.sync.dma_start(xo0[:], xo_d[0])
                xo1 = xtp.tile([128, NPC], bf16, tag="xo1")
                nc.sync.dma_start(xo1[:], xo_d[1])
                cn = xtp.tile([2, N], bf16, tag="cn")
                nc.sync.dma_start(cn[:], cn_d[:])
                ones2 = xtp.tile([2, 128], bf16, tag="ones2")
                nc.gpsimd.memset(ones2[:], 1.0)

                for r in range(RCH):
                    # own-row lhsT: local column c = p*8 + r  ->  [:, r::8]
                    l0 = xo0[:, r::RCH]
                    l1 = xo1[:, r::RCH]
                    for grp in range(2):
                        gts = [psA.tile([128, 512], f32, tag="g", name="g")
                               for _ in range(8)]
                        sls = [slice(grp * 4096 + b * 512,
                                     grp * 4096 + (b + 1) * 512)
                               for b in range(8)]
                        # bank-interleaved: one weight load feeds 8 matmuls,
                        # and accumulation chains of distinct banks overlap
                        for b in range(8):
                            nc.tensor.matmul(gts[b][:], l0, xt0[:, sls[b]],
                                             start=True, stop=False)
                        for b in range(8):
                            nc.tensor.matmul(gts[b][:], l1, xt1[:, sls[b]],
                                             start=False, stop=False)
                        for b in range(8):
                            # rank-2 bf16 hi/lo update adds -(sq_j/2 - eps2/4)
                            nc.tensor.matmul(gts[b][:], ones2[:], cn[:, sls[b]],
                                             start=False, stop=True)
                        for b in range(8):
                            # adj = (g >= rj_i)
                            nc.vector.tensor_scalar(
                                out=adj[r][:, sls[b]], in0=gts[b][:],
                                scalar1=rj[:, r:r + 1], scalar2=None,
                                op0=Alu.is_ge)
                    # degree: free-dim sum on the scalar engine (in-place copy)
                    nc.scalar.activation(
                        adj[r][:], adj[r][:], Act.Identity,
                        bias=0.0, scale=1.0,
                        accum_out=deg_sb[:, r:r + 1])

            # ---------------- core mask + comp0 = core * ord_i ------------
            nc.vector.tensor_scalar(
                out=core01[:], in0=deg_sb[:], scalar1=float(MIN_SAMPLES),
                scalar2=None, op0=Alu.is_ge,
            )
            nc.vector.tensor_mul(comp0[:], core01[:], vv[:])

            # ---------------- phase C: ordinal max propagation ------------
            with (
                tc.tile_pool(name="nbp", bufs=2) as nbp,
                tc.tile_pool(name="scrp", bufs=2) as scrp,
            ):
                comp_cur = comp0
                mh1 = None
                comp_iters = []
                for t in range(1, NITER + 1):
                    # all-gather own chunk -> full ordinal vector
                    agi = dram.tile([128, RCH], bf16, tag=f"agi{t}",
                                    name=f"agi{t}")
                    nc.gpsimd.dma_start(agi[:], comp_cur[:])
                    ago = dram.tile([1, N], bf16, tag=f"ago{t}", name=f"ago{t}",
                                    addr_space="Shared")
                    nc.gpsimd.collective_compute(
                        "AllGather",
                        Alu.bypass,
                        replica_groups=[list(range(NCORES))],
                        ins=[agi[:].opt()],
                        outs=[ago[:].opt()],
                    )

                    mh0 = misc.tile([128, RCH], f32, tag=f"mh0_{t}",
                                    name=f"mh0_{t}")
                    mh = misc.tile([128, RCH], f32, tag=f"mh_{t}",
                                   name=f"mh_{t}")
                    # replicate gathered row to all partitions (one DMA per half)
                    nbs = []
                    for h in range(2):
                        nb = nbp.tile([128, HALF], bf16, tag=f"nb{h}",
                                      name=f"nb{t}_{h}")
                        nc.sync.dma_start(
                            nb[:],
                            ago[0:1, h * HALF:(h + 1) * HALF]
                            .partition_broadcast(128))
                        nbs.append(nb)
                    for h in range(2):
                        for r in range(RCH):
                            scr = scrp.tile([128, HALF], bf16, tag="scr",
                                            name=f"scr{t}_{h}_{r}")
                            nc.vector._custom_dve(
                                maxred,
                                out=scr[:],
                                in0=adj[r][:, h * HALF:(h + 1) * HALF],
                                in1=nbs[h][:],
                                s0=(zero[:, 0:1] if h == 0
                                    else mh0[:, r:r + 1]),
                                accum_out=(mh0[:, r:r + 1] if h == 0
                                           else mh[:, r:r + 1]),
                            )
                    if t == 1:
                        mh1 = mh
                    compn = misc.tile([128, RCH], bf16, tag=f"comp{t}",
                                      name=f"comp{t}")
                    nc.vector.tensor_mul(compn[:], core01[:], mh[:])
                    comp_iters.append(compn)
                    comp_cur = compn

            # ---------------- outputs ------------------------------------
            nc.sync.dma_start(deg_o[:], deg_sb[:])
            nc.sync.dma_start(bord_o[:], mh1[:])
            nc.sync.dma_start(c2_o[:], comp_iters[1][:])
            nc.sync.dma_start(c3_o[:], comp_iters[2][:])

    nc.compile()
    return nc


def _prepare_inputs(X):
    X = np.ascontiguousarray(X, dtype=np.float32)
    sq = np.sum(X * X, axis=1, dtype=np.float32)          # [N]
    # adj  <=>  G >= (sq_i/2 - eps2/4) + (sq_j/2 - eps2/4)
    thr = sq * np.float32(0.5) - np.float32(EPS2 / 4.0)   # [N]

    xt_bf = X.T.astype(ml_dtypes.bfloat16)                # [256, 8192]
    xt = np.ascontiguousarray(xt_bf.reshape(2, 128, N))

    cneg_f = (-thr).astype(np.float32)
    hi = cneg_f.astype(ml_dtypes.bfloat16)
    lo = (cneg_f - hi.astype(np.float32)).astype(ml_dtypes.bfloat16)
    cneg = np.ascontiguousarray(np.stack([hi, lo], axis=0))  # [2, 8192]

    ords_bf = ORDS.astype(ml_dtypes.bfloat16)             # exact
    in_maps = []
    for m in range(NCORES):
        rows = np.arange(m * NPC, (m + 1) * NPC)
        # local i = p*RCH + r  ->  [128, RCH] layout
        rows_pr = rows.reshape(128, RCH)
        in_maps.append({
            "xt": xt,
            "cneg": cneg,
            "xtown": np.ascontiguousarray(xt_bf[:, rows].reshape(2, 128, NPC)),
            "rj": np.ascontiguousarray(thr[rows_pr]),
            "vv": np.ascontiguousarray(ords_bf[rows_pr]),
        })
    return in_maps


def _decode_ords(vals_f32):
    """Map ordinal-encoded f32 values back to indices; 0.0 -> BIG."""
    vals = np.asarray(vals_f32, np.float32)
    bits = vals.astype(ml_dtypes.bfloat16).view(np.uint16).astype(np.int64)
    idx = 0x3F80 - bits
    out = np.where(vals == 0.0, BIG, idx)
    ok = (vals == 0.0) | ((idx >= 0) & (idx < N))
    return out, bool(ok.all())


def _host_finish(deg, bord, comp):
    """Exact numpy port of the reference's label-numbering tail."""
    idx = np.arange(N, dtype=np.int64)
    core = deg >= MIN_SAMPLES
    is_rep = core & (comp == idx)
    cid = np.cumsum(is_rep.astype(np.int64)) - 1
    comp_safe = np.minimum(comp, N - 1)
    core_label = np.where(core, cid[comp_safe], -1)
    first_core_nb = bord
    has_nb = first_core_nb < N
    nb_safe = np.minimum(first_core_nb, N - 1)
    border_label = np.where(has_nb, core_label[nb_safe], -1)
    return np.where(core, core_label, border_label).astype(np.int64)


def _host_fallback(X):
    """Full-precision numpy recomputation (only used if the device
    propagation has not reached the fixpoint, which does not happen)."""
    X = np.asarray(X, dtype=np.float32)
    sq = np.sum(X * X, axis=1, dtype=np.float32)
    G = X @ X.T
    d2 = sq[:, None] + sq[None, :] - 2.0 * G
    adj = d2 <= np.float32(EPS2)
    deg = adj.sum(1)
    core = deg >= MIN_SAMPLES
    idx = np.arange(N, dtype=np.int64)
    comp = np.where(core, idx, BIG)
    adjc = adj & core[None, :]
    while True:
        new = comp.copy()
        for s in range(0, N, 1024):
            cand = np.where(adjc[s:s + 1024], comp[None, :], BIG).min(1)
            new[s:s + 1024] = np.minimum(comp[s:s + 1024], cand)
        new = np.where(core, new, BIG)
        if (new == comp).all():
            break
        comp = new
    bord = np.where(adjc, idx[None, :], BIG).min(1)
    return _host_finish(deg.astype(np.int64), bord, comp)


def _flatten_out(arrs):
    """[8 cores][128, RCH] -> [8192] in global row order."""
    return np.concatenate([np.asarray(a, np.float32).reshape(-1) for a in arrs])


def _run_device(in_maps):
    from concourse import bass_utils
    if "nc" not in _CACHE:
        _CACHE["nc"] = _build_nc()
    res = bass_utils.run_bass_kernel_spmd(
        _CACHE["nc"], in_maps, list(range(NCORES)))
    return res.results


def kernel(X):
    in_maps = _prepare_inputs(X)
    results = _run_device(in_maps)

    deg = _flatten_out([results[m]["deg"] for m in range(NCORES)])
    vbord = _flatten_out([results[m]["bord"] for m in range(NCORES)])
    v2 = _flatten_out([results[m]["comp2"] for m in range(NCORES)])
    v3 = _flatten_out([results[m]["comp3"] for m in range(NCORES)])

    if not np.array_equal(v2, v3):
        return _host_fallback(X)

    comp, ok1 = _decode_ords(v3)
    bord, ok2 = _decode_ords(vbord)
    if not (ok1 and ok2):
        return _host_fallback(X)
    degi = np.rint(deg).astype(np.int64)
    return _host_finish(degi, bord, comp)
